# revision 30
# baseline (speedup 1.0000x reference)
"""Trainium2 Bass kernel for DriverNet: 2-layer LSTM cell (single step, zero
initial state) + linear head over B=1M rows, data-parallel on 8 NeuronCores.

V2 design (vs the 68-75us V1 baseline):
- x is pre-transposed HOST-SIDE into block-lhsT layout [88, nblk/4*128]
  (4 blocks x 22 feats incl. a ones row for the bias), so the 256 PE
  transposes and ~21us of DVE PSUM-evac copies of V1 vanish entirely.
- supertile = 128 blocks (16384 rows). PSUM: one shared "gates" pool holds
  g0 (32 chunks x 60 -> 4 banks) then g1 (8 chunks x 240 -> 4 banks) in
  phase; h1t/h2t transposes + ypre use the rest. The g0(s+1) matmuls'
  WAR on g1(s) is satisfied by program order -> no stall.
- final linear via PE: vp = sigma(o1)*tanh(c2) tiles (with a ones column)
  are PE-transposed to [81, 128] and hit a block-diagonal W_lin (+bias
  row) -> ypre[128, 16] per 16-block group; kills the V1 DVE t-mul +
  1x-rate tensor_reduce (~8us DVE).
- tanh(c2) is a cubic polynomial on DVE (c2 in +-0.55, fit range from a
  host-side sample; max err <1e-3): z2=TT, u=TS(b*z2+c) and the final
  *z merges into the vp product -> only 2 extra DVE ops.
- tanh(c1) (c1 in +-1, quintic poly, 4 extra DVE ops) runs on DVE for
  half the supertiles, on ACT for the rest (TC1_ACT_EVERY=2) - the knob
  that balances ACT vs DVE busy. Both transpose evacs are DVE
  tensor_copy (GPSIMD cannot touch PSUM; DMA refuses PSUM sources).
- emission is a 5-deep software pipeline: iteration k emits
  dma(k+1), A(k)=MM0+sio0/tg0+c1, T1(k-1), B(k-2)=MM1+sio1/tg1+c2+vp,
  T2(k-3), H(k)=tc1+h1, C(k-4)=MMy+ytanh+store, so every cross-engine
  handoff has >= 1 iteration of slack (in-order engines otherwise
  serialize head-of-line; a naive per-super emission measures 110us in
  the cost-model sim vs 60us pipelined).
- cost-model sim: 60.3us (ACT busy 41.3, DVE 40.4, PE 21, DMA 19.3);
  measured ~67.6us HW vs 74.5us for the V1 baseline on this harness.
- nonzero h0/c0 (never produced by the spec) falls back to exact numpy.
"""

import os
import numpy as np
import ml_dtypes

B = 1 << 20
IN_DIM, HID, OUT_DIM = 21, 5, 1
NCORES = 8
BC = B // NCORES          # 131072 rows per core
NBLK = BC // 128          # 1024 blocks per core
L0C = 4                   # blocks per L0 chunk (K = 4*22 = 88)
L1C = 16                  # blocks per L1 chunk (K = 16*6 = 96)
L0_PER_BANK = 8           # 8*60 = 480 <= 512 fp32
L1_PER_BANK = 2           # 2*240 = 480

NB = int(os.environ.get("NB", "96"))          # supertile size in blocks
SUPERS = [NB] * (NBLK // NB) + ([NBLK % NB] if NBLK % NB else [])
NBMAX = max(SUPERS)

_CACHE = {}
LAST_RESULTS = None


def _build_program(reps=1):
    import contextlib
    import concourse.bacc as bacc
    import concourse.tile as tile
    import concourse.mybir as mybir

    AF = mybir.ActivationFunctionType
    ALU = mybir.AluOpType
    BF16 = mybir.dt.bfloat16
    F32 = mybir.dt.float32
    env = lambda k, d: int(os.environ.get(k, d))

    nc = bacc.Bacc("TRN2", target_bir_lowering=False, debug=False, num_devices=NCORES)

    xt_d = nc.declare_dram_parameter("xt", [88, (NBLK // L0C) * 128], BF16, isOutput=False)
    # bf16 consts in one tensor: [ident 128 | w0blk 60 | w1blk 240 | wyblk 16]
    CW = 128 + 60 + 240 + 16
    cp_d = nc.declare_dram_parameter("cpack", [128, CW], BF16, isOutput=False)
    cf_d = nc.declare_dram_parameter("coef", [128, 8], F32, isOutput=False)
    y_d = nc.declare_dram_parameter("y", [BC, 1], F32, isOutput=True)

    TC1_ACT_EVERY = env("TC1_ACT_EVERY", 2)    # every k-th supertile: tc1 on ACT
    TC2_ACT = env("TC2_ACT", 0)
    EVAC1 = os.environ.get("EVAC1", "gpsimd")  # h1t evac engine
    EVAC2 = os.environ.get("EVAC2", "gpsimd")  # h2t evac engine

    with tile.TileContext(nc) as tc:
        with (
            tc.tile_pool(name="const", bufs=1) as constp,
            tc.tile_pool(name="xin", bufs=env("XIN_BUFS", 3)) as xinp,
            tc.tile_pool(name="g0_ps", bufs=env("G0_BUFS", 1), space="PSUM") as g0psp,
            tc.tile_pool(name="g1_ps", bufs=env("G1_BUFS", 1), space="PSUM") as g1psp,
            tc.tile_pool(name="aux_ps", bufs=env("AUX_BUFS", 2), space="PSUM") as auxpsp,
            tc.tile_pool(name="h1t_sb", bufs=env("H1TSB_BUFS", 2)) as h1tsbp,
            tc.tile_pool(name="h2t_sb", bufs=env("H2TSB_BUFS", 2)) as h2tsbp,
            tc.tile_pool(name="acts", bufs=env("ACTS_BUFS", 2)) as actsp,
            tc.tile_pool(name="yout", bufs=env("YOUT_BUFS", 2)) as youtp,
        ):
            cp_sb = constp.tile([128, CW], BF16)
            nc.sync.dma_start(cp_sb[:], cp_d[:])
            id_sb = cp_sb[:, 0:128]
            w0_sb = cp_sb[0:88, 128:188]
            w1_sb = cp_sb[0:96, 188:428]
            wy_sb = cp_sb[0:81, 428:444]
            cf_sb = constp.tile([128, 8], F32)
            nc.gpsimd.dma_start(cf_sb[:], cf_d[:])
            # pre-trigger the sigmoid/tanh ACT table load (~2.7us) so it
            # overlaps the first x-load/matmul instead of stalling sio0
            warm = constp.tile([128, 2], BF16, tag="actwarm")
            nc.scalar.activation(warm[:, 0:1], id_sb[:, 0:1], AF.Sigmoid)
            nc.scalar.activation(warm[:, 1:2], id_sb[:, 0:1], AF.Tanh)

            # persistent ping-pong tiles with memset-once ones slots
            h1_tiles, vp_tiles = [], []
            NCH1M = NBMAX // L1C
            NPP = int(os.environ.get("NPP", "3"))
            for pp in range(NPP):
                ht = constp.tile([128, NCH1M * 96], BF16, tag=f"h1tile{pp}")
                nc.vector.memset(
                    ht[:].rearrange("p (r f) -> p r f", f=6)[:, :, 5:6], 1.0
                )
                h1_tiles.append(ht)
                vt = constp.tile([128, NCH1M * 81], BF16, tag=f"vptile{pp}")
                nc.vector.memset(
                    vt[:].rearrange("p (c w) -> p c w", w=81)[:, :, 80:81], 1.0
                )
                vp_tiles.append(vt)

            if reps > 1:
                rep_ctx = tc.For_i(
                    0, reps, 1, hint_engines=tuple(nc.engines),
                    staggered_reset=bool(int(os.environ.get("STAGRESET", "1"))),
                )
            else:
                rep_ctx = contextlib.nullcontext()

            XSPLIT = env("XSPLIT", 1)   # 1: one DMA per g0 bank-group

            def emit_dma(si, nb, s0):
                nch0 = nb // L0C
                cb = (s0 // 128) // L0C
                x_tile = xinp.tile([88, nch0 * 128], BF16, tag="xt")
                grp = (L0_PER_BANK * 128) if XSPLIT else (nch0 * 128)
                for lo in range(0, nch0 * 128, grp):
                    hi = min(lo + grp, nch0 * 128)
                    nc.sync.dma_start(
                        out=x_tile[:, lo:hi],
                        in_=xt_d[:, cb * 128 + lo : cb * 128 + hi],
                    )
                return x_tile

            def emit_A(si, nb, s0, x_tile):
                S = nb * 128
                nch0 = nb // L0C
                nch1 = nb // L1C
                g0b = (nch0 + L0_PER_BANK - 1) // L0_PER_BANK
                g1b = (nch1 + L1_PER_BANK - 1) // L1_PER_BANK
                tc1_act = TC1_ACT_EVERY > 0 and (si % TC1_ACT_EVERY) == 0

                # ---- L0 gates ----
                g0_ps = g0psp.tile([128, g0b * 512], F32, tag="g0")
                for c in range(nch0):
                    off = (c // L0_PER_BANK) * 512 + (c % L0_PER_BANK) * 60
                    nc.tensor.matmul(
                        g0_ps[:, off : off + 60],
                        x_tile[:, c * 128 : (c + 1) * 128],
                        w0_sb[:],
                        start=True,
                        stop=True,
                    )
                g0v = (
                    g0_ps[:]
                    .rearrange("p (b x) -> p b x", x=512)[:, :, : L0_PER_BANK * 60]
                    .rearrange("p b (c n) -> p b c n", n=60)
                )
                sio0 = actsp.tile([128, nch0 * 40], BF16, tag="sio0")
                nc.scalar.activation(
                    sio0[:].rearrange("p (b c n) -> p b c n", n=40, c=L0_PER_BANK),
                    g0v[:, :, :, 0:40],
                    AF.Sigmoid,
                )
                tg0 = actsp.tile([128, nb * HID], BF16, tag="tg0")
                nc.scalar.activation(
                    tg0[:].rearrange("p (b c n) -> p b c n", n=20, c=L0_PER_BANK),
                    g0v[:, :, :, 40:60],
                    AF.Tanh,
                )
                sio0v = sio0[:].rearrange("p (c n) -> p c n", n=40)
                c1 = actsp.tile([128, nb * HID], BF16, tag="c1")
                nc.vector.tensor_mul(
                    c1[:].rearrange("p (c n) -> p c n", n=20),
                    sio0v[:, :, 0:20],
                    tg0[:].rearrange("p (c n) -> p c n", n=20),
                )
                return dict(
                    si=si, nb=nb, s0=s0, nch1=nch1, g1b=g1b,
                    sio0v=sio0v, c1=c1, tc1_act=tc1_act,
                )

            def emit_H(ctx):
                si, nb, nch1 = ctx["si"], ctx["nb"], ctx["nch1"]
                sio0v, c1 = ctx["sio0v"], ctx["c1"]
                h1 = h1_tiles[si % NPP]
                # h1 slots are 6-wide (5 + ones); view at 4-block-chunk grain
                h1w = (
                    h1[:, : nch1 * 96]
                    .rearrange("p (r f) -> p r f", f=6)[:, :, 0:5]
                    .rearrange("p (c d) f -> p c d f", d=L0C)
                )
                so0 = sio0v[:, :, 20:40].rearrange("p c (d f) -> p c d f", f=HID)
                if ctx["tc1_act"]:
                    tc1 = actsp.tile([128, nb * HID], BF16, tag="tc1")
                    nc.scalar.activation(tc1[:], c1[:], AF.Tanh)
                    nc.vector.tensor_mul(
                        h1w,
                        so0,
                        tc1[:].rearrange("p (c d f) -> p c d f", d=L0C, f=HID),
                    )
                else:
                    # tanh(z) ~= z*(a*z2*z2 + b*z2 + c): h1 = so*(z*w)
                    z2 = actsp.tile([128, nb * HID], BF16, tag="z2")
                    nc.vector.tensor_mul(z2[:], c1[:], c1[:])
                    u = actsp.tile([128, nb * HID], BF16, tag="u")
                    nc.vector.tensor_scalar(
                        u[:], z2[:], cf_sb[:, 0:1], cf_sb[:, 1:2], ALU.mult, ALU.add
                    )
                    w = actsp.tile([128, nb * HID], BF16, tag="w")
                    nc.vector.tensor_mul(w[:], u[:], z2[:])
                    nc.vector.tensor_scalar(
                        w[:], w[:], cf_sb[:, 2:3], None, ALU.add
                    )
                    t1 = actsp.tile([128, nb * HID], BF16, tag="t1")
                    nc.vector.tensor_mul(
                        t1[:].rearrange("p (c n) -> p c n", n=20),
                        sio0v[:, :, 20:40],
                        c1[:].rearrange("p (c n) -> p c n", n=20),
                    )
                    nc.vector.tensor_mul(
                        h1w,
                        t1[:].rearrange("p (c d f) -> p c d f", d=L0C, f=HID),
                        w[:].rearrange("p (c d f) -> p c d f", d=L0C, f=HID),
                    )
                ctx["h1"] = h1
                return ctx

            def emit_T1(ctx):
                nch1, h1 = ctx["nch1"], ctx["h1"]
                h1t_ps = auxpsp.tile([96, nch1 * 128], BF16, tag="aux")
                for c in range(nch1):
                    nc.tensor.transpose(
                        h1t_ps[:, c * 128 : (c + 1) * 128],
                        h1[:, c * 96 : (c + 1) * 96],
                        id_sb[:],
                    )
                h1t_sb = h1tsbp.tile([96, nch1 * 128], BF16, tag="h1tsb")
                getattr(nc, EVAC1).tensor_copy(h1t_sb[:], h1t_ps[:])
                ctx["h1t_sb"] = h1t_sb
                return ctx

            def emit_B(ctx):
                si, nb, s0 = ctx["si"], ctx["nb"], ctx["s0"]
                nch1, g1b = ctx["nch1"], ctx["g1b"]
                h1t_sb = ctx["h1t_sb"]
                g1_ps = g1psp.tile([128, g1b * 512], F32, tag="g1")
                for c in range(nch1):
                    off = (c // L1_PER_BANK) * 512 + (c % L1_PER_BANK) * 240
                    nc.tensor.matmul(
                        g1_ps[:, off : off + 240],
                        h1t_sb[:, c * 128 : (c + 1) * 128],
                        w1_sb[:],
                        start=True,
                        stop=True,
                    )
                g1v = (
                    g1_ps[:]
                    .rearrange("p (b x) -> p b x", x=512)[:, :, : L1_PER_BANK * 240]
                    .rearrange("p b (c n) -> p b c n", n=240)
                )
                sio1 = actsp.tile([128, nch1 * 160], BF16, tag="sio1")
                nc.scalar.activation(
                    sio1[:].rearrange("p (b c n) -> p b c n", n=160, c=L1_PER_BANK),
                    g1v[:, :, :, 0:160],
                    AF.Sigmoid,
                )
                tg1 = actsp.tile([128, nb * HID], BF16, tag="tg1")
                nc.scalar.activation(
                    tg1[:].rearrange("p (b c n) -> p b c n", n=80, c=L1_PER_BANK),
                    g1v[:, :, :, 160:240],
                    AF.Tanh,
                )
                sio1v = sio1[:].rearrange("p (c n) -> p c n", n=160)
                c2 = actsp.tile([128, nb * HID], BF16, tag="c2")
                nc.vector.tensor_mul(
                    c2[:].rearrange("p (c n) -> p c n", n=80),
                    sio1v[:, :, 0:80],
                    tg1[:].rearrange("p (c n) -> p c n", n=80),
                )
                vp = vp_tiles[si % NPP]
                vpw = vp[:, : nch1 * 81].rearrange("p (c w) -> p c w", w=81)[
                    :, :, 0:80
                ]
                if TC2_ACT:
                    tc2 = actsp.tile([128, nb * HID], BF16, tag="tc2")
                    nc.scalar.activation(tc2[:], c2[:], AF.Tanh)
                    nc.vector.tensor_mul(
                        vpw,
                        sio1v[:, :, 80:160],
                        tc2[:].rearrange("p (c n) -> p c n", n=80),
                    )
                else:
                    z2b = actsp.tile([128, nb * HID], BF16, tag="z2b")
                    nc.vector.tensor_mul(z2b[:], c2[:], c2[:])
                    ub = actsp.tile([128, nb * HID], BF16, tag="ub")
                    nc.vector.tensor_scalar(
                        ub[:], z2b[:], cf_sb[:, 3:4], cf_sb[:, 4:5], ALU.mult, ALU.add
                    )
                    t1b = actsp.tile([128, nb * HID], BF16, tag="t1b")
                    nc.vector.tensor_mul(
                        t1b[:].rearrange("p (c n) -> p c n", n=80),
                        sio1v[:, :, 80:160],
                        c2[:].rearrange("p (c n) -> p c n", n=80),
                    )
                    nc.vector.tensor_mul(
                        vpw,
                        t1b[:].rearrange("p (c n) -> p c n", n=80),
                        ub[:].rearrange("p (c n) -> p c n", n=80),
                    )

                ctx["vp"] = vp
                return ctx

            def emit_T2(ctx):
                nch1, vp = ctx["nch1"], ctx["vp"]
                h2t_ps = auxpsp.tile([81, nch1 * 128], BF16, tag="aux")
                for c in range(nch1):
                    nc.tensor.transpose(
                        h2t_ps[:, c * 128 : (c + 1) * 128],
                        vp[:, c * 81 : (c + 1) * 81],
                        id_sb[:],
                    )
                h2t_sb = h2tsbp.tile([81, nch1 * 128], BF16, tag="h2tsb")
                getattr(nc, EVAC2).tensor_copy(h2t_sb[:], h2t_ps[:])
                ctx["h2t_sb"] = h2t_sb
                return ctx

            def emit_C(ctx):
                si, nb, s0 = ctx["si"], ctx["nb"], ctx["s0"]
                nch1 = ctx["nch1"]
                h2t_sb = ctx["h2t_sb"]
                S = nb * 128
                # host row assignment is partition-major (row = s0 + p*nb + b),
                # so batch-on-partitions ypre gives a contiguous y store
                ypre_ps = auxpsp.tile([128, nch1 * 16], F32, tag="aux")
                for c in range(nch1):
                    nc.tensor.matmul(
                        ypre_ps[:, c * 16 : (c + 1) * 16],
                        h2t_sb[:, c * 128 : (c + 1) * 128],
                        wy_sb[:],
                        start=True,
                        stop=True,
                    )
                y_tile = youtp.tile([128, nb], F32, tag="y")
                nc.scalar.activation(y_tile[:], ypre_ps[:], AF.Tanh)

                def store():
                    eng = nc.scalar if env("Y_ON_ACT", 0) else nc.sync
                    eng.dma_start(
                        out=y_d[s0 : s0 + S, 0:1].rearrange(
                            "(p r) o -> p (r o)", p=128
                        ),
                        in_=y_tile[:],
                    )

                return store

            with rep_ctx:
                nS = len(SUPERS)
                offs = [sum(SUPERS[:i]) * 128 for i in range(nS)]
                xts = [None] * (nS + 5)
                ctxs = [None] * (nS + 5)
                stores = [None] * (nS + 5)
                STAGES = set(os.environ.get("STAGES", "a,t1,b,t2,h,c").split(","))
                xts[0] = emit_dma(0, SUPERS[0], offs[0])
                for k in range(nS + 5):
                    if k + 1 < nS:
                        xts[k + 1] = emit_dma(k + 1, SUPERS[k + 1], offs[k + 1])
                    if k < nS:
                        ctxs[k] = emit_A(k, SUPERS[k], offs[k], xts[k])
                    if "t1" in STAGES and 1 <= k <= nS:
                        ctxs[k - 1] = emit_T1(ctxs[k - 1])
                    if "b" in STAGES and 2 <= k <= nS + 1:
                        ctxs[k - 2] = emit_B(ctxs[k - 2])
                    if "t2" in STAGES and 3 <= k <= nS + 2:
                        ctxs[k - 3] = emit_T2(ctxs[k - 3])
                    if "h" in STAGES and k < nS:
                        ctxs[k] = emit_H(ctxs[k])
                    if "c" in STAGES and 4 <= k <= nS + 3:
                        stores[k - 4] = emit_C(ctxs[k - 4])
                        stores[k - 4]()
    nc.compile()
    return nc


def _fit_tanh_poly(lo, deg):
    zz = np.linspace(-lo, lo, 4001)
    t = np.tanh(zz)
    A = np.stack([zz ** (2 * k + 1) for k in range(deg)], 1)
    coef, *_ = np.linalg.lstsq(A, t, rcond=None)
    w = np.ones_like(zz)
    for _ in range(60):
        r = A @ coef - t
        w = 0.9 * w + 0.1 * (np.abs(r) + 1e-9)
        coef, *_ = np.linalg.lstsq(A * w[:, None], t * w, rcond=None)
    return coef  # lowest power first


def _build_inputs(x, W_ih0, W_hh0, b_ih0, b_hh0, W_ih1, W_hh1, b_ih1, b_hh1, W_lin, b_lin):
    bf16 = ml_dtypes.bfloat16
    b0 = (np.asarray(b_ih0) + np.asarray(b_hh0)).astype(np.float32)
    b1 = (np.asarray(b_ih1) + np.asarray(b_hh1)).astype(np.float32)
    W0 = np.asarray(W_ih0, np.float32)
    W1 = np.asarray(W_ih1, np.float32)
    WL = np.asarray(W_lin, np.float32)
    bL = np.asarray(b_lin, np.float32)
    x = np.asarray(x, np.float32)
    sel = {"i": range(0, 5), "g": range(10, 15), "o": range(15, 20)}

    def blockdiag(W, b, chunk, slot):
        kin = W.shape[1]
        out = np.zeros((chunk * slot, chunk * 15), np.float32)
        for dr in range(chunk):
            for grp, key in enumerate(("i", "o", "g")):
                for kk, gr in enumerate(sel[key]):
                    col = grp * (chunk * 5) + dr * 5 + kk
                    r0 = dr * slot
                    out[r0 : r0 + kin, col] = W[gr, :]
                    out[r0 + kin, col] = b[gr]
        return out.astype(bf16)

    w0blk = blockdiag(W0, b0, L0C, 22)
    w1blk = blockdiag(W1, b1, L1C, 6)
    wyblk = np.zeros((81, 16), np.float32)
    for n in range(16):
        wyblk[n * 5 : n * 5 + 5, n] = WL[0]
        wyblk[80, n] = bL[0]
    CW = 128 + 60 + 240 + 16
    cpack = np.zeros((128, CW), bf16)
    cpack[:, 0:128] = np.eye(128, dtype=bf16)
    cpack[0:88, 128:188] = w0blk
    cpack[0:96, 188:428] = w1blk
    cpack[0:81, 428:444] = wyblk.astype(bf16)

    # sample-range fit for the tc2 cubic; full [-1,1] for the tc1 quintic
    sig = lambda v: 1.0 / (1.0 + np.exp(-v))
    xs = x[:: max(1, B // 65536)]
    gs = xs @ W0.T + b0
    h1s = sig(gs[:, 15:20]) * np.tanh(sig(gs[:, 0:5]) * np.tanh(gs[:, 10:15]))
    g1s = h1s @ W1.T + b1
    c2s = sig(g1s[:, 0:5]) * np.tanh(g1s[:, 10:15])
    r2 = min(1.0, float(np.abs(c2s).max()) * 1.15 + 0.02)
    q5 = _fit_tanh_poly(1.0, 3)   # [c, b, a] lowest-first
    q3 = _fit_tanh_poly(r2, 2)
    coef = np.zeros((128, 8), np.float32)
    coef[:, 0] = q5[2]  # a5 (z^5)
    coef[:, 1] = q5[1]  # b5 (z^3)
    coef[:, 2] = q5[0]  # c5 (z)
    coef[:, 3] = q3[1]  # b3
    coef[:, 4] = q3[0]  # c3

    # host transpose to block-lhsT layout with PARTITION-MAJOR row order:
    # within supertile s (rows s0..s0+nb*128), partition p holds rows
    # s0 + p*nb + b; chunk c covers blocks b = 4c..4c+3.
    # xt[22*d + f, 128*(chunk_base+c) + p] = x[s0 + p*nb + 4c + d, f]
    xb = x.astype(bf16)
    nch_tot = NBLK // L0C
    in_maps = []
    for cidx in range(NCORES):
        xc = xb[cidx * BC : (cidx + 1) * BC]
        arr = np.empty((4, 22, nch_tot, 128), bf16)
        arr[:, IN_DIM] = bf16(1.0)
        s0 = 0
        cb = 0
        for nb in ([NB] * (NBLK // NB) + ([NBLK % NB] if NBLK % NB else [])):
            nch = nb // L0C
            src = xc[s0 : s0 + nb * 128].reshape(128, nch, 4, IN_DIM)
            arr[:, :IN_DIM, cb : cb + nch, :] = src.transpose(2, 3, 1, 0)
            s0 += nb * 128
            cb += nch
        xt = arr.reshape(88, nch_tot * 128)
        in_maps.append({"xt": xt, "cpack": cpack, "coef": coef})
    return in_maps


def _reference_numpy(x, h0, c0, W_ih0, W_hh0, b_ih0, b_hh0, W_ih1, W_hh1, b_ih1, b_hh1, W_lin, b_lin):
    # general fallback (never taken for the spec'd zero-state inputs)
    def cell(x_, h, c, Wi, Wh, bi, bh):
        g = x_ @ Wi.T + h @ Wh.T + (bi + bh)
        i, f, gg, o = np.split(g, 4, axis=-1)
        sig = lambda z: 1.0 / (1.0 + np.exp(-z))
        cn = sig(f) * c + sig(i) * np.tanh(gg)
        return sig(o) * np.tanh(cn), cn

    h1, _ = cell(x, h0[0], c0[0], W_ih0, W_hh0, b_ih0, b_hh0)
    h2, _ = cell(h1, h0[1], c0[1], W_ih1, W_hh1, b_ih1, b_hh1)
    return np.tanh(h2 @ W_lin.T + b_lin).astype(np.float32)


def kernel(x, h0, c0, W_ih0, W_hh0, b_ih0, b_hh0, W_ih1, W_hh1, b_ih1, b_hh1, W_lin, b_lin):
    global LAST_RESULTS
    args = dict(
        x=np.asarray(x), h0=np.asarray(h0), c0=np.asarray(c0),
        W_ih0=np.asarray(W_ih0), W_hh0=np.asarray(W_hh0),
        b_ih0=np.asarray(b_ih0), b_hh0=np.asarray(b_hh0),
        W_ih1=np.asarray(W_ih1), W_hh1=np.asarray(W_hh1),
        b_ih1=np.asarray(b_ih1), b_hh1=np.asarray(b_hh1),
        W_lin=np.asarray(W_lin), b_lin=np.asarray(b_lin),
    )
    if np.any(args["h0"]) or np.any(args["c0"]):
        return _reference_numpy(**args)

    from concourse.bass_utils import run_bass_kernel_spmd

    if "nc" not in _CACHE:
        _CACHE["nc"] = _build_program()
    nc = _CACHE["nc"]

    in_maps = _build_inputs(
        args["x"], args["W_ih0"], args["W_hh0"], args["b_ih0"], args["b_hh0"],
        args["W_ih1"], args["W_hh1"], args["b_ih1"], args["b_hh1"],
        args["W_lin"], args["b_lin"],
    )
    trace = bool(int(os.environ.get("TRN_TRACE", "0")))
    res = run_bass_kernel_spmd(nc, in_maps, list(range(NCORES)), trace=trace)
    LAST_RESULTS = res
    return np.concatenate([res.results[i]["y"] for i in range(NCORES)], axis=0)
